# revision 7
# baseline (speedup 1.0000x reference)
"""CityMoE (top-2 of 8 experts + shared expert) Trainium2 kernel.

Strategy: token-shard the fused (B*S*N)=8192 tokens across 8 NeuronCores
(1024 tokens/core). Each core runs the router and evaluates all 8 routed
experts densely with the sparse top-2 combine weights folded into the
SwiGLU intermediate (mathematically identical to gather/dispatch), plus
the sigmoid-gated shared expert. No collectives: per-core outputs are
disjoint token slices; the host concatenates.

Layout: activations are kept feature-major on chip (xT: [H, T]) so all
matmuls contract over the partition axis. Weights are pre-transposed and
cast to fp16 on the host (offline weight packing); matmuls are fp16 with
fp32 PSUM accumulation. The router runs in fp32 for top-2 stability.
"""

import numpy as np

N_CORES = 8
T_FULL = 8192
H = 512
E = 8
I_MOE = 1024
I_SH = 2048
T_C = T_FULL // N_CORES     # tokens per core
TBLK = 512                  # matmul moving width (tokens per block)
NBLK = T_C // TBLK          # 2 blocks per core
NT = T_C // 128             # 8 token tiles per core

_COMPILED = None
LAST_RESULT = None


def _build_program():
    import concourse.bacc as bacc
    import concourse.mybir as mybir
    import concourse.tile as tile

    f32 = mybir.dt.float32
    f16 = mybir.dt.float16
    Act = mybir.ActivationFunctionType
    Alu = mybir.AluOpType
    X = mybir.AxisListType.X

    nc = bacc.Bacc("TRN2", target_bir_lowering=False, debug=False,
                   num_devices=N_CORES)

    x_ap = nc.dram_tensor("x", [T_C, H], f32, kind="ExternalInput").ap()
    g9_ap = nc.dram_tensor("gate9t", [4, 128, 9], f32, kind="ExternalInput").ap()
    w1_ap = nc.dram_tensor("w1t", [E, 4, 128, I_MOE], f16, kind="ExternalInput").ap()
    w3_ap = nc.dram_tensor("w3t", [E, 4, 128, I_MOE], f16, kind="ExternalInput").ap()
    w2_ap = nc.dram_tensor("w2t", [E, 8, 128, H], f16, kind="ExternalInput").ap()
    sw1_ap = nc.dram_tensor("sw1t", [4, 128, I_SH], f16, kind="ExternalInput").ap()
    sw3_ap = nc.dram_tensor("sw3t", [4, 128, I_SH], f16, kind="ExternalInput").ap()
    sw2_ap = nc.dram_tensor("sw2t", [16, 128, H], f16, kind="ExternalInput").ap()
    id_ap = nc.dram_tensor("ident", [128, 128], f32, kind="ExternalInput").ap()

    outt_ap = nc.dram_tensor("outt", [H, T_C], f32, kind="ExternalOutput").ap()
    log_ap = nc.dram_tensor("logits", [T_C, E], f32, kind="ExternalOutput").ap()

    from contextlib import ExitStack

    with tile.TileContext(nc) as tc, ExitStack() as ctx:
        ep = ctx.enter_context

        # ---------- pools ----------
        pconst = ep(tc.tile_pool(name="const", bufs=1))
        pxt16 = ep(tc.tile_pool(name="xt16", bufs=1))
        pcomb = ep(tc.tile_pool(name="comb", bufs=1))
        pxin = ep(tc.tile_pool(name="xin", bufs=2))
        px32 = ep(tc.tile_pool(name="x32", bufs=2))
        pasm = ep(tc.tile_pool(name="asm", bufs=2))
        pbc = ep(tc.tile_pool(name="bc", bufs=1))
        pw13 = ep(tc.tile_pool(name="w13", bufs=2))
        pw2 = ep(tc.tile_pool(name="w2", bufs=2))
        psw13 = ep(tc.tile_pool(name="sw13", bufs=1))
        psw2 = ep(tc.tile_pool(name="sw2", bufs=1))
        ph16p = ep(tc.tile_pool(name="h16", bufs=28))
        psilu = ep(tc.tile_pool(name="silu", bufs=2))
        posb = ep(tc.tile_pool(name="osb", bufs=2))
        # PSUM: pph (4 banks, shared tag) + ppo (4 accumulators) = 8 banks.
        pph = ep(tc.tile_pool(name="pph", bufs=4, space="PSUM"))
        ppo = ep(tc.tile_pool(name="ppo", bufs=1, space="PSUM"))

        # ---------- constants ----------
        ident = pconst.tile([128, 128], f32, name="ident")
        nc.sync.dma_start(ident[:], id_ap[:, :])
        gate_sb = []
        for k in range(4):
            g = pconst.tile([128, 9], f32, name=f"g{k}")
            nc.sync.dma_start(g[:], g9_ap[k, :, :])
            gate_sb.append(g)

        # xT16[k]: fp16 transposed activations, chunk k of H (128 rows),
        # all T_C token columns.
        xt16 = [pxt16.tile([128, T_C], f16, name=f"xt{k}") for k in range(4)]
        # comb_rows[e]: per-expert combine weight row (e<8), shared gate (e=8),
        # each a single-partition tile so partition_broadcast can read it.
        comb_rows = [pcomb.tile([1, T_C], f16, name=f"cr{e}") for e in range(9)]

        # ---------- phase A: transpose x, router, top-2 combine ----------
        for g in range(NT):
            ts = slice(g * 128, (g + 1) * 128)
            xin = pxin.tile([128, H], f32, name="xin")
            nc.sync.dma_start(xin[:], x_ap[ts, :])

            trps = pph.tile([128, 512], f32, name="ph")
            for k in range(4):
                nc.tensor.transpose(
                    trps[:, k * 128:(k + 1) * 128],
                    xin[:, k * 128:(k + 1) * 128], ident[:])

            x32 = []
            for k in range(4):
                t = px32.tile([128, 128], f32, name=f"x32_{k}")
                nc.vector.tensor_copy(t[:], trps[:, k * 128:(k + 1) * 128])
                x32.append(t)
                nc.scalar.copy(xt16[k][:, ts], trps[:, k * 128:(k + 1) * 128])

            logps_t = pph.tile([128, 512], f32, name="ph")
            logps = logps_t[:, 0:9]
            for k in range(4):
                nc.tensor.matmul(logps, x32[k][:], gate_sb[k][:],
                                 start=(k == 0), stop=(k == 3))

            lsb = pasm.tile([128, 8], f32, name="lsb")
            nc.vector.tensor_copy(lsb[:], logps_t[:, 0:8])
            nc.sync.dma_start(log_ap[ts, :], lsb[:])

            # softmax (no max-subtraction needed: |logits| < ~4) + top-2
            eu = pasm.tile([128, 8], f32, name="eu")
            nc.scalar.activation(eu[:], logps_t[:, 0:8], Act.Exp)
            ssum = pasm.tile([128, 1], f32, name="ssum")
            nc.vector.reduce_sum(ssum[:], eu[:], axis=X)
            rs = pasm.tile([128, 1], f32, name="rs")
            nc.vector.reciprocal(rs[:], ssum[:])
            m1 = pasm.tile([128, 1], f32, name="m1")
            nc.vector.reduce_max(m1[:], eu[:], axis=X)
            mlt = pasm.tile([128, 8], f32, name="mlt")
            nc.vector.tensor_scalar(mlt[:], eu[:], m1[:], None, Alu.is_lt)
            eu2 = pasm.tile([128, 8], f32, name="eu2")
            nc.vector.tensor_tensor(eu2[:], eu[:], mlt[:], op=Alu.mult)
            m2 = pasm.tile([128, 1], f32, name="m2")
            nc.vector.reduce_max(m2[:], eu2[:], axis=X)
            msk = pasm.tile([128, 8], f32, name="msk")
            nc.vector.tensor_scalar(msk[:], eu[:], m2[:], None, Alu.is_ge)
            cw = pasm.tile([128, 8], f32, name="cw")
            nc.vector.tensor_tensor(cw[:], eu[:], msk[:], op=Alu.mult)
            comb9 = pasm.tile([128, 9], f32, name="comb9")
            nc.vector.tensor_scalar(comb9[:, 0:8], cw[:], rs[:], None, Alu.mult)
            nc.scalar.activation(comb9[:, 8:9], logps_t[:, 8:9], Act.Sigmoid)

            # Transpose each combine column separately so every result lands
            # at partition 0 (engines cannot address partition offsets >0).
            cgrp = [pph.tile([128, 512], f32, name="ph") for _ in range(3)]
            for e in range(9):
                j, sl = divmod(e, 4)
                nc.tensor.transpose(cgrp[j][0:1, sl * 128:(sl + 1) * 128],
                                    comb9[:, e:e + 1], ident[:])
            for e in range(9):
                j, sl = divmod(e, 4)
                nc.vector.tensor_copy(comb_rows[e][0:1, ts],
                                      cgrp[j][0:1, sl * 128:(sl + 1) * 128])

        # ---------- shared-expert weights (persist across blocks) ----------
        sw1 = []
        sw3 = []
        for k in range(4):
            t1 = psw13.tile([128, I_SH], f16, name=f"sw1_{k}")
            nc.sync.dma_start(t1[:], sw1_ap[k, :, :])
            sw1.append(t1)
            t3 = psw13.tile([128, I_SH], f16, name=f"sw3_{k}")
            nc.sync.dma_start(t3[:], sw3_ap[k, :, :])
            sw3.append(t3)
        sw2 = []
        for i in range(16):
            t = psw2.tile([128, H], f16, name=f"sw2_{i}")
            nc.sync.dma_start(t[:], sw2_ap[i, :, :])
            sw2.append(t)

        # ---------- phase B: expert FFNs, combine folded into h ----------
        for b in range(NBLK):
            cs = slice(b * TBLK, (b + 1) * TBLK)
            outps = [ppo.tile([128, TBLK], f32, name=f"po{h}")
                     for h in range(4)]
            bc = []
            for e in range(9):
                t = pbc.tile([128, TBLK], f16, name=f"bc{e}")
                nc.gpsimd.partition_broadcast(t[:], comb_rows[e][0:1, cs])
                bc.append(t)

            started = [False] * 4

            def ffn(w1t, w3t, w2t, n_i, bce, last):
                h16s = []
                for i in range(n_i):
                    isl = slice(i * 128, (i + 1) * 128)
                    h1 = pph.tile([128, TBLK], f32, name="ph")
                    for k in range(4):
                        nc.tensor.matmul(h1[:], w1t[k][:, isl], xt16[k][:, cs],
                                         start=(k == 0), stop=(k == 3))
                    h3 = pph.tile([128, TBLK], f32, name="ph")
                    for k in range(4):
                        nc.tensor.matmul(h3[:], w3t[k][:, isl], xt16[k][:, cs],
                                         start=(k == 0), stop=(k == 3))
                    s1 = psilu.tile([128, TBLK], f16, name="s1")
                    nc.scalar.activation(s1[:], h1[:], Act.Silu)
                    hb = psilu.tile([128, TBLK], f16, name="hb")
                    nc.vector.tensor_tensor(hb[:], s1[:], h3[:], op=Alu.mult)
                    h16 = ph16p.tile([128, TBLK], f16, name="h16")
                    nc.vector.tensor_tensor(h16[:], hb[:], bce[:], op=Alu.mult)
                    h16s.append(h16)
                for h in range(4):
                    hsl = slice(h * 128, (h + 1) * 128)
                    for i in range(n_i):
                        nc.tensor.matmul(
                            outps[h][:], w2t[i][:, hsl], h16s[i][:],
                            start=(not started[h]),
                            stop=(last and i == n_i - 1))
                        started[h] = True

            for e in range(E):
                w1 = []
                w3 = []
                for k in range(4):
                    t1 = pw13.tile([128, I_MOE], f16, name=f"w1_{k}")
                    nc.sync.dma_start(t1[:], w1_ap[e, k, :, :])
                    w1.append(t1)
                    t3 = pw13.tile([128, I_MOE], f16, name=f"w3_{k}")
                    nc.sync.dma_start(t3[:], w3_ap[e, k, :, :])
                    w3.append(t3)
                w2 = []
                for i in range(8):
                    t = pw2.tile([128, H], f16, name=f"w2_{i}")
                    nc.sync.dma_start(t[:], w2_ap[e, i, :, :])
                    w2.append(t)
                ffn(w1, w3, w2, 8, bc[e], last=False)

            ffn(sw1, sw3, sw2, 16, bc[8], last=True)

            for h in range(4):
                osb = posb.tile([128, TBLK], f32, name="osb")
                nc.vector.tensor_copy(osb[:], outps[h][:])
                nc.sync.dma_start(outt_ap[h * 128:(h + 1) * 128, cs], osb[:])

    nc.compile()
    return nc


def _get_compiled():
    global _COMPILED
    if _COMPILED is None:
        _COMPILED = _build_program()
    return _COMPILED


def _prep_inputs(hidden_states, gate_w, expert_w1, expert_w2, expert_w3,
                 shared_w1, shared_w2, shared_w3, shared_gate_w):
    f16 = np.float16
    f32 = np.float32
    x = np.ascontiguousarray(np.asarray(hidden_states, dtype=f32).reshape(T_FULL, H))
    gate_w = np.asarray(gate_w, dtype=f32)
    shared_gate_w = np.asarray(shared_gate_w, dtype=f32)
    g9 = np.concatenate([gate_w.T, shared_gate_w.T], axis=1)  # [H, 9]
    g9 = np.ascontiguousarray(g9.reshape(4, 128, 9))

    w1t = np.ascontiguousarray(
        np.asarray(expert_w1, dtype=f32).transpose(0, 2, 1).reshape(E, 4, 128, I_MOE).astype(f16))
    w3t = np.ascontiguousarray(
        np.asarray(expert_w3, dtype=f32).transpose(0, 2, 1).reshape(E, 4, 128, I_MOE).astype(f16))
    w2t = np.ascontiguousarray(
        np.asarray(expert_w2, dtype=f32).transpose(0, 2, 1).reshape(E, 8, 128, H).astype(f16))
    sw1t = np.ascontiguousarray(
        np.asarray(shared_w1, dtype=f32).T.reshape(4, 128, I_SH).astype(f16))
    sw3t = np.ascontiguousarray(
        np.asarray(shared_w3, dtype=f32).T.reshape(4, 128, I_SH).astype(f16))
    sw2t = np.ascontiguousarray(
        np.asarray(shared_w2, dtype=f32).T.reshape(16, 128, H).astype(f16))
    ident = np.eye(128, dtype=f32)

    shared = {
        "gate9t": g9, "w1t": w1t, "w3t": w3t, "w2t": w2t,
        "sw1t": sw1t, "sw3t": sw3t, "sw2t": sw2t, "ident": ident,
    }
    in_maps = []
    for c in range(N_CORES):
        m = dict(shared)
        m["x"] = np.ascontiguousarray(x[c * T_C:(c + 1) * T_C])
        in_maps.append(m)
    return in_maps


def kernel(hidden_states, gate_w, expert_w1, expert_w2, expert_w3,
           shared_w1, shared_w2, shared_w3, shared_gate_w):
    global LAST_RESULT
    from concourse.bass_utils import run_bass_kernel_spmd

    nc = _get_compiled()
    in_maps = _prep_inputs(hidden_states, gate_w, expert_w1, expert_w2,
                           expert_w3, shared_w1, shared_w2, shared_w3,
                           shared_gate_w)
    res = run_bass_kernel_spmd(nc, in_maps, core_ids=list(range(N_CORES)))
    LAST_RESULT = res

    out = np.empty((T_FULL, H), dtype=np.float32)
    logits = np.empty((T_FULL, E), dtype=np.float32)
    for c in range(N_CORES):
        out[c * T_C:(c + 1) * T_C] = res.results[c]["outt"].T
        logits[c * T_C:(c + 1) * T_C] = res.results[c]["logits"]
    B, S, N = 1, 8, 1024
    return out.reshape(B, S, N, H), logits
